# revision 14
# baseline (speedup 1.0000x reference)
"""Trainium2 Bass kernel for GQA attention (B=2, T=2048, C=4096, H=32, KV=8, D=128)
with RoPE and causal mask.

Sharding: tensor-parallel over heads across 8 cores. Each core owns 4 Q heads and
their shared KV head: projects q/k/v for those heads, runs causal attention, and
computes a partial output projection; the host sums the 8 partials.

All on-chip layouts are transposed ([feature, token]) so every matmul consumes
natural slices:
  qT/kT/vT = W^T @ x  via lhsT=W-tile [128c, cols], rhs=xT-tile [128c, 512t]
  sT[tk, tq] = kT-tile^T @ qT-chunk   (per 128-row key tile x 512-col query chunk)
  pT = exp(sT/sqrt(D) - 10) on ACT; strictly-causal-upper tiles skipped entirely
  yT[d, tq] += v-tile^T @ pT          (v pre-transposed to [t, d] via PE transpose)
  out[tq, :] += yT_h^T @ wo_h         (accumulate 4 heads in PSUM, evict, DMA out)

Projection PSUM banks are evicted raw (ACT bf16 copies, fast) and RoPE is applied
afterwards on SBUF bf16 by the DVE in 2x mode, off the critical path. Diagonal
score tiles stream only their causally-valid columns; their exp outputs land in
ring buffers whose masked column prefix is zeroed once at startup, so the attn@v
and denominator matmuls can stay full-width. The softmax denominator sums pairs
of p tiles on DVE first, halving the ones-matmul passes on the PE. Output
partials are written in bf16 and summed on the host in f32.
"""

import os
from collections import deque
from contextlib import ExitStack

import numpy as np
import ml_dtypes

import concourse.bacc as bacc
import concourse.mybir as mybir
import concourse.tile as tile

BF = mybir.dt.bfloat16
F32 = mybir.dt.float32
AFT = mybir.ActivationFunctionType

NCORES = 8
B, T, C = 2, 2048, 4096
H, KV, D = 32, 8, 128
QH = H // NCORES          # 4 q-heads per core
CT = C // 128             # 32 contraction tiles
NCH = T // 512            # 4 query chunks per batch
SCALE = 1.0 / float(np.sqrt(D))
EXP_BIAS = -10.0
ROPE_BASE = 10000.0
SKEW = 8                  # q matmuls trail k/v by this many c-tiles

bf16 = ml_dtypes.bfloat16


def emit_program():
    nc = bacc.Bacc("TRN2", target_bir_lowering=False, debug=False,
                   num_devices=NCORES)

    xT_d = nc.dram_tensor("xT", [C, B * T], BF, kind="ExternalInput").ap()
    wq_d = nc.dram_tensor("wq", [C, QH * D], BF, kind="ExternalInput").ap()
    wk_d = nc.dram_tensor("wk", [C, D], BF, kind="ExternalInput").ap()
    wv_d = nc.dram_tensor("wv", [C, D], BF, kind="ExternalInput").ap()
    wo_d = nc.dram_tensor("woA", [128, QH, C], BF, kind="ExternalInput").ap()
    cos_d = nc.dram_tensor("cosT", [D, T], BF, kind="ExternalInput").ap()
    sin_d = nc.dram_tensor("sinTr", [D, T], BF, kind="ExternalInput").ap()
    alw_d = nc.dram_tensor("allowA", [128, 4, 512], BF, kind="ExternalInput").ap()
    id_d = nc.dram_tensor("ident", [128, 128], BF, kind="ExternalInput").ap()
    out_d = nc.dram_tensor("out", [B * T, C], BF, kind="ExternalOutput").ap()

    with tile.TileContext(nc) as tc, ExitStack() as ctx:
        const = ctx.enter_context(tc.tile_pool(name="const", bufs=1))
        act = ctx.enter_context(tc.tile_pool(name="act", bufs=1))
        work = ctx.enter_context(tc.tile_pool(name="work", bufs=1))

        # k/v weight tiles lead each group so the first projection matmuls wait
        # on the smallest possible transfer; wo/ident are issued later from the
        # scalar queue so they don't contend with chunk-0/1 activation loads
        wq_sb = const.tile([128, CT, QH * D], BF)
        wk_sb = const.tile([128, CT, D], BF)
        wv_sb = const.tile([128, CT, D], BF)
        wqr = wq_d.rearrange("(ci p) n -> p ci n", p=128)
        wkr = wk_d.rearrange("(ci p) n -> p ci n", p=128)
        wvr = wv_d.rearrange("(ci p) n -> p ci n", p=128)
        # first micro-group rides the (faster) sync queue so the very first
        # k/v matmuls start ASAP; the rest stream on the gpsimd queue
        nc.sync.dma_start(wk_sb[:, 0:2, :], wkr[:, 0:2, :])
        nc.sync.dma_start(wv_sb[:, 0:2, :], wvr[:, 0:2, :])
        nc.sync.dma_start(wq_sb[:, 0:2, :], wqr[:, 0:2, :])
        for g0, g1 in [(2, 4), (4, 12), (12, 20), (20, 28), (28, 32)]:
            s = slice(g0, g1)
            nc.gpsimd.dma_start(wk_sb[:, s, :], wkr[:, s, :])
            nc.gpsimd.dma_start(wv_sb[:, s, :], wvr[:, s, :])
            nc.gpsimd.dma_start(wq_sb[:, s, :], wqr[:, s, :])
        cos_sb = const.tile([D, T], BF)
        sin_sb = const.tile([D, T], BF)
        alw_sb = const.tile([128, 4, 512], BF)
        id_sb = const.tile([128, 128], BF)
        wo_sb = const.tile([128, QH, C], BF)
        onesbf_sb = const.tile([128, 128], BF)
        nc.gpsimd.memset(onesbf_sb[:], 1.0)
        bias_sb = const.tile([128, 1], F32)
        nc.gpsimd.memset(bias_sb[:], EXP_BIAS)

        # exp outputs for diagonal score tiles keep a permanently-zero column
        # prefix: zero every ring buffer once, then only write cols >= 128*o
        PTO_BUFS = 3
        for o in (1, 2, 3):
            for _ in range(PTO_BUFS):
                t = work.tile([128, 512], BF, tag=f"pto{o}", bufs=PTO_BUFS,
                              name=f"pto{o}")
                nc.gpsimd.memset(t[:], 0.0)

        def rope_sbuf(dst, raw, cs):
            # dst = raw * cos + swap_halves(raw) * sin_rot   (bf16 SBUF in/out)
            sw = work.tile([128, 512], BF, tag="sw", bufs=3, name="sw")
            nc.vector.tensor_copy(sw[0:64, :], raw[64:128, :])
            nc.vector.tensor_copy(sw[64:128, :], raw[0:64, :])
            nc.vector.tensor_mul(sw[:], sw[:], sin_sb[:, cs])
            cst = work.tile([128, 512], BF, tag="cst", bufs=3, name="cst")
            nc.vector.tensor_mul(cst[:], raw[:], cos_sb[:, cs])
            nc.vector.tensor_add(dst, cst[:], sw[:])

        # out-projection PSUM tiles live in their own pool so wo_jobs can run
        # in any phase (they only touch these 2 banks + SBUF)
        pops = ctx.enter_context(tc.tile_pool(name="pops", bufs=1, space="PSUM"))
        wo_jobs = deque()

        def make_wo_job(b, j, tl, o, yts):
            def job():
                ops = pops.tile([128, 512], F32, tag="ops", bufs=2, name="ops")
                for h in range(QH):
                    nc.tensor.matmul(
                        ops[:], yts[h][:, 128 * tl:128 * (tl + 1)],
                        wo_sb[:, h, 512 * o:512 * (o + 1)],
                        start=h == 0, stop=h == QH - 1)
                ob = work.tile([128, 512], BF, tag="ob", bufs=4, name="ob")
                if (tl + o) % 2:
                    nc.scalar.copy(ob[:], ops[:])
                else:
                    nc.vector.tensor_copy(ob[:], ops[:])
                r0 = b * T + 512 * j + 128 * tl
                nc.sync.dma_start(out_d[r0:r0 + 128, 512 * o:512 * (o + 1)],
                                  ob[:])
            return job

        for b in range(B):
            qT = act.tile([D, QH, T], BF, tag="qT", name="qT")
            kT = act.tile([D, T], BF, tag="kT", name="kT")
            vT = act.tile([D, T], BF, tag="vT", name="vT")
            vsb = act.tile([128, T // 128, D], BF, tag="v", name="vsb")

            # ---- projections ----
            with tc.tile_pool(name=f"pproj{b}", bufs=1, space="PSUM") as pp:
                for jc in range(NCH):
                    pq = [pp.tile([128, 512], F32, tag=f"pq{h}", name=f"pq{h}")
                          for h in range(QH)]
                    pk = pp.tile([128, 512], F32, tag="pk", name="pk")
                    pv = pp.tile([128, 512], F32, tag="pv", name="pv")
                    xts = {}
                    col0 = b * T + 512 * jc

                    def q_mms(cq):
                        for h in range(QH):
                            nc.tensor.matmul(
                                pq[h][:], wq_sb[:, cq, 128 * h:128 * (h + 1)],
                                xts[cq][:], start=cq == 0, stop=cq == CT - 1)
                        if cq >= SKEW:
                            del xts[cq - SKEW]

                    for ci in range(CT):
                        xt = work.tile([128, 512], BF, tag="xt", bufs=20, name="xt")
                        xts[ci] = xt
                        nc.sync.dma_start(
                            xt[:], xT_d[128 * ci:128 * (ci + 1), col0:col0 + 512])
                        st, sp = ci == 0, ci == CT - 1
                        nc.tensor.matmul(pk[:], wk_sb[:, ci, :], xt[:],
                                         start=st, stop=sp)
                        nc.tensor.matmul(pv[:], wv_sb[:, ci, :], xt[:],
                                         start=st, stop=sp)
                        if ci >= SKEW:
                            q_mms(ci - SKEW)
                    for cq in range(CT - SKEW, CT):
                        q_mms(cq)

                    cs = slice(512 * jc, 512 * (jc + 1))
                    # fast raw PSUM->SBUF bf16 evictions (free banks quickly);
                    # k/v first since the next chunk's k/v matmuls reuse those
                    # banks first, q evictions split ACT/DVE to finish before
                    # the skewed q matmuls need their banks. RoPE runs later on
                    # DVE in 2x mode, off the critical path.
                    kraw = work.tile([128, 512], BF, tag="kraw", bufs=2,
                                     name="kraw")
                    nc.scalar.copy(kraw[:], pk[:])
                    nc.scalar.copy(vT[:, cs], pv[:])
                    qraw = []
                    for h in range(QH):
                        qr = work.tile([128, 512], BF, tag="qraw", bufs=8,
                                       name="qraw")
                        if h < 2:
                            nc.scalar.copy(qr[:], pq[h][:])
                        else:
                            nc.vector.tensor_copy(qr[:], pq[h][:])
                        qraw.append(qr)
                    if b == 0 and jc == 0:
                        # deferred weight loads: issue once chunk 0 is off HBM
                        nc.scalar.dma_start(cos_sb[:], cos_d)
                        nc.scalar.dma_start(sin_sb[:], sin_d)
                        nc.scalar.dma_start(id_sb[:], id_d)
                        nc.scalar.dma_start(wo_sb[:], wo_d)
                        nc.scalar.dma_start(alw_sb[:], alw_d)
                    rope_sbuf(kT[:, cs], kraw, cs)
                    for h in range(QH):
                        rope_sbuf(qT[:, h, cs], qraw[h], cs)

            # ---- transpose v to [t, d] tiles ----
            with tc.tile_pool(name=f"ptr{b}", bufs=1, space="PSUM") as ptr:
                for k in range(T // 128):
                    tp = ptr.tile([128, 128], BF, tag="tp", bufs=4, name="tp")
                    nc.tensor.transpose(tp[:], vT[:, 128 * k:128 * (k + 1)],
                                        id_sb[:])
                    nc.scalar.copy(vsb[:, k, :], tp[:])
                    if k % 2 == 1 and wo_jobs:
                        wo_jobs.popleft()()

            # ---- attention + output projection ----
            with tc.tile_pool(name=f"pattn{b}", bufs=1, space="PSUM") as pa:
                for j in range(NCH):
                    yts = {}
                    for h in range(QH):
                        yps = pa.tile([128, 512], F32, tag="yps", bufs=1,
                                      name="yps")
                        dps = pa.tile([128, 512], F32, tag="dps", bufs=1,
                                      name="dps")
                        K = 4 * j + 4
                        # single pass: scores stream through sps slots, exp
                        # trails on ACT (diagonal tiles column-trimmed), attn@v
                        # accumulates as each pt lands; a 2-level DVE add tree
                        # feeds one denominator matmul per 4 tiles. Jobs pop
                        # between score issue and accumulation so the PE stays
                        # fed while ACT works through the quad's exps.
                        pts = []
                        for qd in range(K // 4):
                            for k in range(4 * qd, 4 * qd + 4):
                                o = k - 4 * j
                                sps = pa.tile([128, 512], F32, tag="sps",
                                              bufs=4, name="sps")
                                c0 = 128 * o if o > 0 else 0
                                nc.tensor.matmul(
                                    sps[:, c0:512],
                                    kT[:, 128 * k:128 * (k + 1)],
                                    qT[:, h, 512 * j + c0:512 * (j + 1)],
                                    start=True, stop=True)
                                if o > 0:
                                    pt = work.tile([128, 512], BF,
                                                   tag=f"pto{o}", bufs=PTO_BUFS,
                                                   name=f"pto{o}")
                                else:
                                    pt = work.tile([128, 512], BF, tag="pt",
                                                   bufs=10, name="pt")
                                nc.scalar.activation(pt[:, c0:512],
                                                     sps[:, c0:512], AFT.Exp,
                                                     bias=bias_sb[:],
                                                     scale=SCALE)
                                if o >= 0:
                                    m = slice(128 * o, 128 * (o + 1))
                                    nc.vector.tensor_mul(pt[:, m], pt[:, m],
                                                         alw_sb[:, o, m])
                                pts.append(pt)
                            npop = 2 if len(wo_jobs) > 4 else 1
                            for _ in range(min(npop, len(wo_jobs))):
                                wo_jobs.popleft()()
                            p2a = work.tile([128, 512], BF, tag="p2", bufs=4,
                                            name="p2")
                            nc.vector.tensor_add(p2a[:], pts[4 * qd][:],
                                                 pts[4 * qd + 1][:])
                            p2b = work.tile([128, 512], BF, tag="p2", bufs=4,
                                            name="p2")
                            nc.vector.tensor_add(p2b[:], pts[4 * qd + 2][:],
                                                 pts[4 * qd + 3][:])
                            p4 = work.tile([128, 512], BF, tag="p4", bufs=3,
                                           name="p4")
                            nc.vector.tensor_add(p4[:], p2a[:], p2b[:])
                            for k in range(4 * qd, 4 * qd + 4):
                                o = k - 4 * j
                                c0 = 128 * o if o > 0 else 0
                                nc.tensor.matmul(yps[:, c0:512], vsb[:, k, :],
                                                 pts[k][:, c0:512],
                                                 start=k == 0, stop=k == K - 1,
                                                 skip_group_check=True)
                            nc.tensor.matmul(dps[:], onesbf_sb[:], p4[:],
                                             start=qd == 0, stop=qd == K // 4 - 1)
                        rec = work.tile([128, 512], F32, tag="rec", bufs=2,
                                        name="rec")
                        nc.vector.reciprocal_approx_fast(rec[:], dps[:])
                        yt = work.tile([128, 512], BF, tag="yt", bufs=8,
                                       name="yt")
                        nc.vector.tensor_mul(yt[:], yps[:], rec[:])
                        yts[h] = yt
                    for tl in range(4):
                        for o in range(C // 512):
                            wo_jobs.append(make_wo_job(b, j, tl, o, yts))
                # keep a few jobs to warm the next batch's attention start;
                # the final batch drains fully
                keep = 0 if b == B - 1 else 16
                while len(wo_jobs) > keep:
                    wo_jobs.popleft()()

    nc.compile()
    return nc


def host_prep(inputs):
    x = np.asarray(inputs["x"], np.float32)
    mask = np.asarray(inputs["mask"], np.float32)
    wq = np.asarray(inputs["wq"], np.float32)
    wk = np.asarray(inputs["wk"], np.float32)
    wv = np.asarray(inputs["wv"], np.float32)
    wo = np.asarray(inputs["wo"], np.float32)

    xT = np.ascontiguousarray(x.reshape(B * T, C).T).astype(bf16)
    inv = 1.0 / (ROPE_BASE ** (np.arange(0, D, 2, dtype=np.float64) / D))
    freqs = np.arange(T, dtype=np.float64)[:, None] * inv[None, :] * B
    emb = np.concatenate([freqs, freqs], axis=-1)       # [T, D]
    cosT = np.cos(emb).T.astype(np.float32).astype(bf16)
    sinT = np.sin(emb).T.astype(np.float32)
    sinT[: D // 2] *= -1.0
    sinTr = sinT.astype(bf16)
    # allow[p, o, jj] = 1 - mask[jj, 128*o + p]  (from the actual mask input)
    allowA = np.ascontiguousarray(
        np.stack([(1.0 - mask[0:512, 128 * o:128 * (o + 1)]).T
                  for o in range(4)], axis=1)).astype(bf16)   # [128, 4, 512]
    ident = np.eye(128, dtype=np.float32).astype(bf16)

    common = dict(xT=xT, cosT=cosT, sinTr=sinTr, allowA=allowA, ident=ident)
    in_maps = []
    for c in range(NCORES):
        m = dict(common)
        m["wq"] = np.ascontiguousarray(wq[:, 512 * c:512 * (c + 1)]).astype(bf16)
        m["wk"] = np.ascontiguousarray(wk[:, 128 * c:128 * (c + 1)]).astype(bf16)
        m["wv"] = np.ascontiguousarray(wv[:, 128 * c:128 * (c + 1)]).astype(bf16)
        m["woA"] = np.ascontiguousarray(
            wo[512 * c:512 * (c + 1), :].reshape(QH, 128, C)
            .transpose(1, 0, 2)).astype(bf16)
        in_maps.append(m)
    return in_maps


def kernel(**inputs) -> np.ndarray:
    from concourse.bass_utils import run_bass_kernel_spmd

    in_maps = host_prep(inputs)
    nc = emit_program()
    trace = bool(os.environ.get("BASS_KERNEL_TRACE"))
    res = run_bass_kernel_spmd(nc, in_maps, core_ids=list(range(NCORES)),
                               trace=trace)
    if trace and res.exec_time_ns is not None:
        print(f"HW exec time: {res.exec_time_ns} ns")
        if res.instructions_and_trace is not None:
            print("trace:", res.instructions_and_trace[1])
    total = np.zeros((B * T, C), np.float32)
    for r in res.results:
        total += r["out"].astype(np.float32)
    return total.reshape(B, T, C)


# revision 16
# speedup vs baseline: 1.1574x; 1.1574x over previous
"""Trainium2 Bass kernel for GQA attention (B=2, T=2048, C=4096, H=32, KV=8, D=128)
with RoPE and causal mask.

Sharding: tensor-parallel over heads across 8 cores. Each core owns 4 Q heads and
their shared KV head: projects q/k/v for those heads, runs causal attention, and
computes a partial output projection; the host sums the 8 partials.

All on-chip layouts are transposed ([feature, token]) so every matmul consumes
natural slices:
  qT/kT/vT = W^T @ x  via lhsT=W-tile [128c, cols], rhs=xT-tile [128c, 512t]
  sT[tk, tq] = kT-tile^T @ qT-chunk   (per 128-row key tile x 512-col query chunk)
  pT = exp(sT/sqrt(D) - 10) on ACT; strictly-causal-upper tiles skipped entirely
  yT[d, tq] += v-tile^T @ pT          (v pre-transposed to [t, d] via PE transpose)
  out[tq, :] += yT_h^T @ wo_h         (accumulate 4 heads in PSUM, evict, DMA out)

Projection PSUM banks are evicted raw (ACT bf16 copies, fast) and RoPE is applied
afterwards on SBUF bf16 by the DVE in 2x mode, off the critical path. Diagonal
score tiles stream only their causally-valid columns; their exp outputs land in
ring buffers whose masked column prefix is zeroed once at startup, so the attn@v
and denominator matmuls can stay full-width. The softmax denominator sums pairs
of p tiles on DVE first, halving the ones-matmul passes on the PE. Output
partials are written in bf16 and summed on the host in f32.
"""

import os
from collections import deque
from contextlib import ExitStack

import numpy as np
import ml_dtypes

import concourse.bacc as bacc
import concourse.mybir as mybir
import concourse.tile as tile

BF = mybir.dt.bfloat16
F32 = mybir.dt.float32
AFT = mybir.ActivationFunctionType

NCORES = 8
B, T, C = 2, 2048, 4096
H, KV, D = 32, 8, 128
QH = H // NCORES          # 4 q-heads per core
CT = C // 128             # 32 contraction tiles
NCH = T // 512            # 4 query chunks per batch
SCALE = 1.0 / float(np.sqrt(D))
EXP_BIAS = -10.0
ROPE_BASE = 10000.0
SKEW = 8                  # q matmuls trail k/v by this many c-tiles

bf16 = ml_dtypes.bfloat16


def emit_program():
    nc = bacc.Bacc("TRN2", target_bir_lowering=False, debug=False,
                   num_devices=NCORES)

    xT_d = nc.dram_tensor("xT", [C, B * T], BF, kind="ExternalInput").ap()
    wq_d = nc.dram_tensor("wq", [C, QH * D], BF, kind="ExternalInput").ap()
    wk_d = nc.dram_tensor("wk", [C, D], BF, kind="ExternalInput").ap()
    wv_d = nc.dram_tensor("wv", [C, D], BF, kind="ExternalInput").ap()
    wo_d = nc.dram_tensor("woA", [128, QH, C], BF, kind="ExternalInput").ap()
    cos_d = nc.dram_tensor("cosT", [D, T], BF, kind="ExternalInput").ap()
    sin_d = nc.dram_tensor("sinTr", [D, T], BF, kind="ExternalInput").ap()
    alw_d = nc.dram_tensor("allowA", [128, 4, 512], BF, kind="ExternalInput").ap()
    id_d = nc.dram_tensor("ident", [128, 128], BF, kind="ExternalInput").ap()
    out_d = nc.dram_tensor("out", [B * T, C], BF, kind="ExternalOutput").ap()

    with tile.TileContext(nc) as tc, ExitStack() as ctx:
        const = ctx.enter_context(tc.tile_pool(name="const", bufs=1))
        act = ctx.enter_context(tc.tile_pool(name="act", bufs=1))
        work = ctx.enter_context(tc.tile_pool(name="work", bufs=1))

        # k/v weight tiles lead each group so the first projection matmuls wait
        # on the smallest possible transfer; wo/ident are issued later from the
        # scalar queue so they don't contend with chunk-0/1 activation loads
        wq_sb = const.tile([128, CT, QH * D], BF)
        wk_sb = const.tile([128, CT, D], BF)
        wv_sb = const.tile([128, CT, D], BF)
        wqr = wq_d.rearrange("(ci p) n -> p ci n", p=128)
        wkr = wk_d.rearrange("(ci p) n -> p ci n", p=128)
        wvr = wv_d.rearrange("(ci p) n -> p ci n", p=128)
        # first micro-group rides the (faster) sync queue so the very first
        # k/v matmuls start ASAP; the rest stream on the gpsimd queue
        nc.sync.dma_start(wk_sb[:, 0:2, :], wkr[:, 0:2, :])
        nc.sync.dma_start(wv_sb[:, 0:2, :], wvr[:, 0:2, :])
        nc.sync.dma_start(wq_sb[:, 0:2, :], wqr[:, 0:2, :])
        for g0, g1 in [(2, 4), (4, 12), (12, 20), (20, 28), (28, 32)]:
            s = slice(g0, g1)
            nc.gpsimd.dma_start(wk_sb[:, s, :], wkr[:, s, :])
            nc.gpsimd.dma_start(wv_sb[:, s, :], wvr[:, s, :])
            nc.gpsimd.dma_start(wq_sb[:, s, :], wqr[:, s, :])
        cos_sb = const.tile([D, T], BF)
        sin_sb = const.tile([D, T], BF)
        alw_sb = const.tile([128, 4, 512], BF)
        id_sb = const.tile([128, 128], BF)
        wo_sb = const.tile([128, QH, C], BF)
        onesbf_sb = const.tile([128, 128], BF)
        nc.gpsimd.memset(onesbf_sb[:], 1.0)
        bias_sb = const.tile([128, 1], F32)
        nc.gpsimd.memset(bias_sb[:], EXP_BIAS)

        # exp outputs for diagonal score tiles keep a permanently-zero column
        # prefix: zero every ring buffer once, then only write cols >= 128*o
        PTO_BUFS = 3
        for o in (1, 2, 3):
            for _ in range(PTO_BUFS):
                t = work.tile([128, 512], BF, tag=f"pto{o}", bufs=PTO_BUFS,
                              name=f"pto{o}")
                nc.gpsimd.memset(t[:], 0.0)

        def rope_sbuf(dst, raw, cs):
            # dst = raw * cos + swap_halves(raw) * sin_rot   (bf16 SBUF in/out)
            sw = work.tile([128, 512], BF, tag="sw", bufs=3, name="sw")
            nc.vector.tensor_copy(sw[0:64, :], raw[64:128, :])
            nc.vector.tensor_copy(sw[64:128, :], raw[0:64, :])
            nc.vector.tensor_mul(sw[:], sw[:], sin_sb[:, cs])
            cst = work.tile([128, 512], BF, tag="cst", bufs=3, name="cst")
            nc.vector.tensor_mul(cst[:], raw[:], cos_sb[:, cs])
            nc.vector.tensor_add(dst, cst[:], sw[:])

        # out-projection PSUM tiles live in their own pool so wo_jobs can run
        # in any phase (they only touch these 2 banks + SBUF)
        pops = ctx.enter_context(tc.tile_pool(name="pops", bufs=1, space="PSUM"))
        wo_jobs = deque()

        def make_wo_job(b, j, tl, o, yts):
            def job():
                ops = pops.tile([128, 512], F32, tag="ops", bufs=2, name="ops")
                for h in range(QH):
                    nc.tensor.matmul(
                        ops[:], yts[h][:, 128 * tl:128 * (tl + 1)],
                        wo_sb[:, h, 512 * o:512 * (o + 1)],
                        start=h == 0, stop=h == QH - 1)
                ob = work.tile([128, 512], BF, tag="ob", bufs=4, name="ob")
                nc.vector.tensor_copy(ob[:], ops[:])
                r0 = b * T + 512 * j + 128 * tl
                nc.sync.dma_start(out_d[r0:r0 + 128, 512 * o:512 * (o + 1)],
                                  ob[:])
            return job

        for b in range(B):
            qT = act.tile([D, QH, T], BF, tag="qT", name="qT")
            kT = act.tile([D, T], BF, tag="kT", name="kT")
            vT = act.tile([D, T], BF, tag="vT", name="vT")
            vsb = act.tile([128, T // 128, D], BF, tag="v", name="vsb")

            # ---- projections ----
            with tc.tile_pool(name=f"pproj{b}", bufs=1, space="PSUM") as pp:
                for jc in range(NCH):
                    pq = [pp.tile([128, 512], F32, tag=f"pq{h}", name=f"pq{h}")
                          for h in range(QH)]
                    pk = pp.tile([128, 512], F32, tag="pk", name="pk")
                    pv = pp.tile([128, 512], F32, tag="pv", name="pv")
                    xts = {}
                    col0 = b * T + 512 * jc

                    def q_mms(cq):
                        for h in range(QH):
                            nc.tensor.matmul(
                                pq[h][:], wq_sb[:, cq, 128 * h:128 * (h + 1)],
                                xts[cq][:], start=cq == 0, stop=cq == CT - 1)
                        if cq >= SKEW:
                            del xts[cq - SKEW]

                    for ci in range(CT):
                        xt = work.tile([128, 512], BF, tag="xt", bufs=20, name="xt")
                        xts[ci] = xt
                        nc.sync.dma_start(
                            xt[:], xT_d[128 * ci:128 * (ci + 1), col0:col0 + 512])
                        st, sp = ci == 0, ci == CT - 1
                        nc.tensor.matmul(pk[:], wk_sb[:, ci, :], xt[:],
                                         start=st, stop=sp)
                        nc.tensor.matmul(pv[:], wv_sb[:, ci, :], xt[:],
                                         start=st, stop=sp)
                        if ci >= SKEW:
                            q_mms(ci - SKEW)
                    for cq in range(CT - SKEW, CT):
                        q_mms(cq)

                    cs = slice(512 * jc, 512 * (jc + 1))
                    # fast raw PSUM->SBUF bf16 evictions (free banks quickly);
                    # k/v first since the next chunk's k/v matmuls reuse those
                    # banks first, q evictions split ACT/DVE to finish before
                    # the skewed q matmuls need their banks. RoPE runs later on
                    # DVE in 2x mode, off the critical path.
                    kraw = work.tile([128, 512], BF, tag="kraw", bufs=2,
                                     name="kraw")
                    nc.scalar.copy(kraw[:], pk[:])
                    nc.scalar.copy(vT[:, cs], pv[:])
                    qraw = []
                    for h in range(QH):
                        qr = work.tile([128, 512], BF, tag="qraw", bufs=8,
                                       name="qraw")
                        if h < 2:
                            nc.scalar.copy(qr[:], pq[h][:])
                        else:
                            nc.vector.tensor_copy(qr[:], pq[h][:])
                        qraw.append(qr)
                    if b == 0 and jc == 0:
                        # deferred weight loads: issue once chunk 0 is off HBM
                        nc.scalar.dma_start(cos_sb[:], cos_d)
                        nc.scalar.dma_start(sin_sb[:], sin_d)
                        nc.scalar.dma_start(id_sb[:], id_d)
                        nc.scalar.dma_start(wo_sb[:], wo_d)
                        nc.scalar.dma_start(alw_sb[:], alw_d)
                    rope_sbuf(kT[:, cs], kraw, cs)
                    for h in range(QH):
                        rope_sbuf(qT[:, h, cs], qraw[h], cs)

            # ---- transpose v to [t, d] tiles ----
            with tc.tile_pool(name=f"ptr{b}", bufs=1, space="PSUM") as ptr:
                for k in range(T // 128):
                    tp = ptr.tile([128, 128], BF, tag="tp", bufs=4, name="tp")
                    nc.tensor.transpose(tp[:], vT[:, 128 * k:128 * (k + 1)],
                                        id_sb[:])
                    nc.scalar.copy(vsb[:, k, :], tp[:])
                    if k % 2 == 1 and wo_jobs:
                        wo_jobs.popleft()()

            # ---- attention + output projection ----
            with tc.tile_pool(name=f"pattn{b}", bufs=1, space="PSUM") as pa:
                for j in range(NCH):
                    yts = {}
                    for h in range(QH):
                        yps = pa.tile([128, 512], F32, tag="yps", bufs=1,
                                      name="yps")
                        dps = pa.tile([128, 512], F32, tag="dps", bufs=1,
                                      name="dps")
                        K = 4 * j + 4
                        # single pass: scores stream through sps slots, exp
                        # trails on ACT (diagonal tiles column-trimmed), attn@v
                        # accumulates as each pt lands; a 2-level DVE add tree
                        # feeds one denominator matmul per 4 tiles. Jobs pop
                        # between score issue and accumulation so the PE stays
                        # fed while ACT works through the quad's exps.
                        pts = []
                        for i in range(K // 2):
                            for k in (2 * i, 2 * i + 1):
                                o = k - 4 * j
                                sps = pa.tile([128, 512], F32, tag="sps",
                                              bufs=4, name="sps")
                                c0 = 128 * o if o > 0 else 0
                                nc.tensor.matmul(
                                    sps[:, c0:512],
                                    kT[:, 128 * k:128 * (k + 1)],
                                    qT[:, h, 512 * j + c0:512 * (j + 1)],
                                    start=True, stop=True)
                                if o > 0:
                                    pt = work.tile([128, 512], BF,
                                                   tag=f"pto{o}", bufs=PTO_BUFS,
                                                   name=f"pto{o}")
                                else:
                                    pt = work.tile([128, 512], BF, tag="pt",
                                                   bufs=10, name="pt")
                                nc.scalar.activation(pt[:, c0:512],
                                                     sps[:, c0:512], AFT.Exp,
                                                     bias=bias_sb[:],
                                                     scale=SCALE)
                                if o >= 0:
                                    m = slice(128 * o, 128 * (o + 1))
                                    nc.vector.tensor_mul(pt[:, m], pt[:, m],
                                                         alw_sb[:, o, m])
                                pts.append(pt)
                            npop = 2 if len(wo_jobs) > 8 else 1
                            for _ in range(min(npop, len(wo_jobs))):
                                wo_jobs.popleft()()
                            p2 = work.tile([128, 512], BF, tag="p2", bufs=4,
                                           name="p2")
                            nc.vector.tensor_add(p2[:], pts[2 * i][:],
                                                 pts[2 * i + 1][:])
                            for k in (2 * i, 2 * i + 1):
                                o = k - 4 * j
                                c0 = 128 * o if o > 0 else 0
                                nc.tensor.matmul(yps[:, c0:512], vsb[:, k, :],
                                                 pts[k][:, c0:512],
                                                 start=k == 0, stop=k == K - 1,
                                                 skip_group_check=True)
                            nc.tensor.matmul(dps[:], onesbf_sb[:], p2[:],
                                             start=i == 0, stop=i == K // 2 - 1)
                        rec = work.tile([128, 512], F32, tag="rec", bufs=2,
                                        name="rec")
                        nc.vector.reciprocal_approx_fast(rec[:], dps[:])
                        yt = work.tile([128, 512], BF, tag="yt", bufs=8,
                                       name="yt")
                        nc.vector.tensor_mul(yt[:], yps[:], rec[:])
                        yts[h] = yt
                    for tl in range(4):
                        for o in range(C // 512):
                            wo_jobs.append(make_wo_job(b, j, tl, o, yts))
                # keep a few jobs to warm the next batch's attention start;
                # the final batch drains fully
                keep = 0 if b == B - 1 else 16
                while len(wo_jobs) > keep:
                    wo_jobs.popleft()()

    nc.compile()
    return nc


def host_prep(inputs):
    x = np.asarray(inputs["x"], np.float32)
    mask = np.asarray(inputs["mask"], np.float32)
    wq = np.asarray(inputs["wq"], np.float32)
    wk = np.asarray(inputs["wk"], np.float32)
    wv = np.asarray(inputs["wv"], np.float32)
    wo = np.asarray(inputs["wo"], np.float32)

    xT = np.ascontiguousarray(x.reshape(B * T, C).T).astype(bf16)
    inv = 1.0 / (ROPE_BASE ** (np.arange(0, D, 2, dtype=np.float64) / D))
    freqs = np.arange(T, dtype=np.float64)[:, None] * inv[None, :] * B
    emb = np.concatenate([freqs, freqs], axis=-1)       # [T, D]
    cosT = np.cos(emb).T.astype(np.float32).astype(bf16)
    sinT = np.sin(emb).T.astype(np.float32)
    sinT[: D // 2] *= -1.0
    sinTr = sinT.astype(bf16)
    # allow[p, o, jj] = 1 - mask[jj, 128*o + p]  (from the actual mask input)
    allowA = np.ascontiguousarray(
        np.stack([(1.0 - mask[0:512, 128 * o:128 * (o + 1)]).T
                  for o in range(4)], axis=1)).astype(bf16)   # [128, 4, 512]
    ident = np.eye(128, dtype=np.float32).astype(bf16)

    common = dict(xT=xT, cosT=cosT, sinTr=sinTr, allowA=allowA, ident=ident)
    in_maps = []
    for c in range(NCORES):
        m = dict(common)
        m["wq"] = np.ascontiguousarray(wq[:, 512 * c:512 * (c + 1)]).astype(bf16)
        m["wk"] = np.ascontiguousarray(wk[:, 128 * c:128 * (c + 1)]).astype(bf16)
        m["wv"] = np.ascontiguousarray(wv[:, 128 * c:128 * (c + 1)]).astype(bf16)
        m["woA"] = np.ascontiguousarray(
            wo[512 * c:512 * (c + 1), :].reshape(QH, 128, C)
            .transpose(1, 0, 2)).astype(bf16)
        in_maps.append(m)
    return in_maps


def kernel(**inputs) -> np.ndarray:
    from concourse.bass_utils import run_bass_kernel_spmd

    in_maps = host_prep(inputs)
    nc = emit_program()
    trace = bool(os.environ.get("BASS_KERNEL_TRACE"))
    res = run_bass_kernel_spmd(nc, in_maps, core_ids=list(range(NCORES)),
                               trace=trace)
    if trace and res.exec_time_ns is not None:
        print(f"HW exec time: {res.exec_time_ns} ns")
        if res.instructions_and_trace is not None:
            print("trace:", res.instructions_and_trace[1])
    total = np.zeros((B * T, C), np.float32)
    for r in res.results:
        total += r["out"].astype(np.float32)
    return total.reshape(B, T, C)
